# revision 6
# baseline (speedup 1.0000x reference)
"""Trainium2 Bass kernel: monomials x^a y^b z^c (a+b+c <= 3) for N=2M points.

Data-parallel across 8 NeuronCores; each core gets N/8 = 250k points padded
to 128*F*T. Per tile of 128 x F points:
  in-tile  it [P, F, 3]  (point-major interleaved x,y,z; contiguous load)
  out-tile ot [P, F, 20] (point-major; contiguous 5 MB store)
Columns: 0:1  1:x 2:y 3:z  4:x2 5:xy 6:xz 7:y2 8:yz 9:z2
         10:x3 11:x2y 12:x2z 13:xy2 14:xyz 15:xz2 16:y3 17:y2z 18:yz2 19:z3
DVE (fused, step-0 broadcast in0): deg2 = x*(x,y,z)->4:7, y*(y,z)->7:9,
  z*z->9; deg3 = x*cols4:10->10:16, y*cols7:10->16:19, z*col9->19.
ACT: [x,y,z]->cols1:4, ones->col0, issues out-DMAs. SP: in-DMAs.
Raw bass (no Tile): this walrus rejects >1 sync-wait per instruction, so all
waits are standalone wait_ge ops and DMA sems are per-buffer-slot (at most
one DMA in flight per sem keeps 16*n waits unambiguous).
"""

import sys
from contextlib import ExitStack

if "/opt/trn_rl_repo" not in sys.path:
    sys.path.insert(0, "/opt/trn_rl_repo")

import numpy as np
import concourse.bass as bass
import concourse.mybir as mybir
from concourse.bass_utils import run_bass_kernel_spmd

P = 128
K = 20
N_TOTAL = 2_000_000
N_CORES = 8
N_CORE = N_TOTAL // N_CORES  # 250_000
F = 490
T = 4
B = 3
N_PAD = P * F * T  # 250_880

AF = mybir.ActivationFunctionType
F32 = mybir.dt.float32


def _build(nc: bass.Bass) -> bass.Bass:
    v = nc.declare_dram_parameter("vectors", [N_PAD, 3], F32, isOutput=False)
    o = nc.declare_dram_parameter("out", [N_PAD, K], F32, isOutput=True)
    vr = v.rearrange("(t p f) c -> t p (f c)", p=P, f=F)
    orr = o.rearrange("(t p f) k -> t p (f k)", p=P, f=F)

    with ExitStack() as ctx:
        itb = ctx.enter_context(nc.sbuf_tensor("itb", [P, B * F * 3], F32))
        otb = ctx.enter_context(nc.sbuf_tensor("otb", [P, B * F * K], F32))
        s_in = [ctx.enter_context(nc.semaphore(f"s_in{i}")) for i in range(B)]
        s_out = [ctx.enter_context(nc.semaphore(f"s_out{i}")) for i in range(B)]
        s_v = ctx.enter_context(nc.semaphore("s_v"))
        s_d = ctx.enter_context(nc.semaphore("s_d"))
        s_a = ctx.enter_context(nc.semaphore("s_a"))
        block = ctx.enter_context(nc.Block())

        def it_flat(s):
            return itb.ap()[:, s * F * 3 : (s + 1) * F * 3]

        def ot_flat(s):
            return otb.ap()[:, s * F * K : (s + 1) * F * K]

        def it_view(s):
            return it_flat(s).rearrange("p (f c) -> p f c", c=3)

        def ot_view(s):
            return ot_flat(s).rearrange("p (f k) -> p f k", k=K)

        @block.sync
        def _(sync):
            for t in range(T):
                s = t % B
                if t >= B:
                    # WAR: slot's previous tile fully read by ACT and DVE
                    sync.wait_ge(s_a, t - B + 1)
                    sync.wait_ge(s_v, t - B + 1)
                sync.dma_start(out=it_flat(s), in_=vr[t]).then_inc(s_in[s], 16)

        @block.vector
        def _(vector):
            for t in range(T):
                s = t % B
                n_use = t // B + 1
                itv = it_view(s)
                otv = ot_view(s)
                x = itv[:, :, 0:1]
                y = itv[:, :, 1:2]
                z = itv[:, :, 2:3]
                vector.wait_ge(s_in[s], 16 * n_use)
                if t >= B:
                    # WAR: out-DMA of the tile previously in this slot done
                    vector.wait_ge(s_out[s], 16 * (n_use - 1))
                nc.vector.tensor_mul(
                    otv[:, :, 4:7], x.broadcast_to([P, F, 3]), itv[:, :, 0:3]
                )
                nc.vector.tensor_mul(
                    otv[:, :, 7:9], y.broadcast_to([P, F, 2]), itv[:, :, 1:3]
                )
                nc.vector.tensor_mul(otv[:, :, 9:10], z, z).then_inc(s_d, 1)
                # Same-engine RAW: deg3 reads deg2 through the deep DVE
                # pipeline; wait for the deg2 writes to land.
                vector.wait_ge(s_d, t + 1)
                nc.vector.tensor_mul(
                    otv[:, :, 10:16], x.broadcast_to([P, F, 6]), otv[:, :, 4:10]
                )
                nc.vector.tensor_mul(
                    otv[:, :, 16:19], y.broadcast_to([P, F, 3]), otv[:, :, 7:10]
                )
                nc.vector.tensor_mul(otv[:, :, 19:20], z, otv[:, :, 9:10]).then_inc(
                    s_v, 1
                )

        @block.scalar
        def _(scalar):
            for t in range(T):
                s = t % B
                n_use = t // B + 1
                itv = it_view(s)
                otv = ot_view(s)
                scalar.wait_ge(s_in[s], 16 * n_use)
                if t >= B:
                    scalar.wait_ge(s_out[s], 16 * (n_use - 1))
                nc.scalar.copy(otv[:, :, 1:4], itv[:, :, 0:3])
                nc.scalar.activation(
                    otv[:, :, 0:1], itv[:, :, 0:1], AF.Copy, bias=1.0, scale=0.0
                ).then_inc(s_a, 1)
                # HWDGE trigger does not drain the ACT pipe; wait for our
                # own writes (s_a) and DVE's (s_v) before reading otb.
                scalar.wait_ge(s_a, t + 1)
                scalar.wait_ge(s_v, t + 1)
                scalar.dma_start(out=orr[t], in_=ot_flat(s)).then_inc(s_out[s], 16)
            for s in range(B):
                uses = len([t for t in range(T) if t % B == s])
                if uses:
                    scalar.wait_ge(s_out[s], 16 * uses)

    return nc


_CACHE: dict[str, object] = {}


def _get_nc() -> bass.Bass:
    if "nc" not in _CACHE:
        nc = bass.Bass()
        _build(nc)
        _CACHE["nc"] = nc
    return _CACHE["nc"]  # type: ignore[return-value]


def run_spmd(in_maps, trace=False, **kw):
    return run_bass_kernel_spmd(
        _get_nc(), in_maps, core_ids=list(range(N_CORES)), trace=trace, **kw
    )


def make_in_maps(vectors: np.ndarray):
    vectors = np.ascontiguousarray(np.asarray(vectors, dtype=np.float32))
    assert vectors.shape == (N_TOTAL, 3)
    shards = vectors.reshape(N_CORES, N_CORE, 3)
    in_maps = []
    for i in range(N_CORES):
        buf = np.zeros((N_PAD, 3), dtype=np.float32)
        buf[:N_CORE] = shards[i]
        in_maps.append({"vectors": buf})
    return in_maps


def kernel(vectors: np.ndarray) -> np.ndarray:
    res = run_spmd(make_in_maps(vectors))
    out = np.empty((N_TOTAL, K), dtype=np.float32)
    for i in range(N_CORES):
        out[i * N_CORE : (i + 1) * N_CORE] = res.results[i]["out"][:N_CORE]
    return out


# revision 7
# speedup vs baseline: 1.0154x; 1.0154x over previous
"""Trainium2 Bass kernel: monomials x^a y^b z^c (a+b+c <= 3) for N=2M points.

Data-parallel across 8 NeuronCores; each core gets N/8 = 250k points padded
to 128*F*T. Per tile of 128 x F points:
  in-tile  it [P, F, 3]  (point-major interleaved x,y,z; contiguous load)
  out-tile ot [P, F, 20] (point-major; contiguous store)
Columns: 0:1  1:x 2:y 3:z  4:x2 5:xy 6:xz 7:y2 8:yz 9:z2
         10:x3 11:x2y 12:x2z 13:xy2 14:xyz 15:xz2 16:y3 17:y2z 18:yz2 19:z3
DVE (fused, step-0 broadcast in0): deg2 = x*(x,y,z)->4:7, y*(y,z)->7:9,
  z*z->9; deg3 = x*cols4:10->10:16, y*cols7:10->16:19, z*col9->19.
ACT: [x,y,z]->cols1:4, ones->col0, issues out-DMAs. SP: in-DMAs.

Raw bass (no Tile): this walrus rejects >1 sync-wait per instruction, so all
waits are standalone wait_ge ops. Every tile has its own input slot (all
in-DMAs issued upfront, one sem each); output slots are BO-deep with
per-slot sems (one DMA in flight per sem keeps 16*n waits unambiguous).
"""

import sys
from contextlib import ExitStack

if "/opt/trn_rl_repo" not in sys.path:
    sys.path.insert(0, "/opt/trn_rl_repo")

import numpy as np
import concourse.bass as bass
import concourse.mybir as mybir
from concourse.bass_utils import run_bass_kernel_spmd

P = 128
K = 20
N_TOTAL = 2_000_000
N_CORES = 8
N_CORE = N_TOTAL // N_CORES  # 250_000
F = 245
T = 8
BO = 3
N_PAD = P * F * T  # 250_880

AF = mybir.ActivationFunctionType
F32 = mybir.dt.float32


def build(nc: bass.Bass, n_pts: int, f: int, bo: int = BO) -> bass.Bass:
    t_total = n_pts // (P * f)
    assert t_total * P * f == n_pts

    v = nc.declare_dram_parameter("vectors", [n_pts, 3], F32, isOutput=False)
    o = nc.declare_dram_parameter("out", [n_pts, K], F32, isOutput=True)
    vr = v.rearrange("(t p f) c -> t p (f c)", p=P, f=f)
    orr = o.rearrange("(t p f) k -> t p (f k)", p=P, f=f)

    with ExitStack() as ctx:
        itb = ctx.enter_context(nc.sbuf_tensor("itb", [P, t_total * f * 3], F32))
        otb = ctx.enter_context(nc.sbuf_tensor("otb", [P, bo * f * K], F32))
        s_in = [ctx.enter_context(nc.semaphore(f"s_in{i}")) for i in range(t_total)]
        s_out = [ctx.enter_context(nc.semaphore(f"s_out{i}")) for i in range(bo)]
        s_v = ctx.enter_context(nc.semaphore("s_v"))
        s_d = ctx.enter_context(nc.semaphore("s_d"))
        s_a = ctx.enter_context(nc.semaphore("s_a"))
        block = ctx.enter_context(nc.Block(no_gpsimd_drain=True))

        def it_view(t):
            return itb.ap()[:, t * f * 3 : (t + 1) * f * 3].rearrange(
                "p (f c) -> p f c", c=3
            )

        def ot_flat(s):
            return otb.ap()[:, s * f * K : (s + 1) * f * K]

        def ot_view(s):
            return ot_flat(s).rearrange("p (f k) -> p f k", k=K)

        @block.sync
        def _(sync):
            # Each tile has its own input slot: no WAR hazards, issue all
            # loads back-to-back; the SP HWDGE ring drains them FIFO.
            for t in range(t_total):
                sync.dma_start(
                    out=itb.ap()[:, t * f * 3 : (t + 1) * f * 3], in_=vr[t]
                ).then_inc(s_in[t], 16)

        @block.vector
        def _(vector):
            for t in range(t_total):
                s = t % bo
                n_use = t // bo  # completed uses of this out slot
                itv = it_view(t)
                otv = ot_view(s)
                x = itv[:, :, 0:1]
                y = itv[:, :, 1:2]
                z = itv[:, :, 2:3]
                vector.wait_ge(s_in[t], 16)
                if t >= bo:
                    # WAR: out-DMA of the tile previously in this slot done
                    vector.wait_ge(s_out[s], 16 * n_use)
                nc.vector.tensor_mul(
                    otv[:, :, 4:7], x.broadcast_to([P, f, 3]), itv[:, :, 0:3]
                )
                nc.vector.tensor_mul(
                    otv[:, :, 7:9], y.broadcast_to([P, f, 2]), itv[:, :, 1:3]
                )
                nc.vector.tensor_mul(otv[:, :, 9:10], z, z).then_inc(s_d, 1)
                # Same-engine RAW: deg3 reads deg2 through the deep DVE
                # pipeline; wait for the deg2 writes to land.
                vector.wait_ge(s_d, t + 1)
                nc.vector.tensor_mul(
                    otv[:, :, 10:16], x.broadcast_to([P, f, 6]), otv[:, :, 4:10]
                )
                nc.vector.tensor_mul(
                    otv[:, :, 16:19], y.broadcast_to([P, f, 3]), otv[:, :, 7:10]
                )
                nc.vector.tensor_mul(otv[:, :, 19:20], z, otv[:, :, 9:10]).then_inc(
                    s_v, 1
                )

        @block.scalar
        def _(scalar):
            for t in range(t_total):
                s = t % bo
                n_use = t // bo
                itv = it_view(t)
                otv = ot_view(s)
                scalar.wait_ge(s_in[t], 16)
                if t >= bo:
                    scalar.wait_ge(s_out[s], 16 * n_use)
                nc.scalar.copy(otv[:, :, 1:4], itv[:, :, 0:3])
                nc.scalar.activation(
                    otv[:, :, 0:1], itv[:, :, 0:1], AF.Copy, bias=1.0, scale=0.0
                ).then_inc(s_a, 1)
                # HWDGE trigger does not drain the ACT pipe; wait for our
                # own writes (s_a) and DVE's (s_v) before reading otb.
                scalar.wait_ge(s_a, t + 1)
                scalar.wait_ge(s_v, t + 1)
                scalar.dma_start(out=orr[t], in_=ot_flat(s)).then_inc(s_out[s], 16)
            for s in range(bo):
                uses = len([t for t in range(t_total) if t % bo == s])
                if uses:
                    scalar.wait_ge(s_out[s], 16 * uses)

    return nc


_CACHE: dict[str, object] = {}


def _get_nc() -> bass.Bass:
    if "nc" not in _CACHE:
        nc = bass.Bass()
        build(nc, N_PAD, F, BO)
        _CACHE["nc"] = nc
    return _CACHE["nc"]  # type: ignore[return-value]


def run_spmd(in_maps, trace=False, **kw):
    return run_bass_kernel_spmd(
        _get_nc(), in_maps, core_ids=list(range(N_CORES)), trace=trace, **kw
    )


def make_in_maps(vectors: np.ndarray):
    vectors = np.ascontiguousarray(np.asarray(vectors, dtype=np.float32))
    assert vectors.shape == (N_TOTAL, 3)
    shards = vectors.reshape(N_CORES, N_CORE, 3)
    in_maps = []
    for i in range(N_CORES):
        buf = np.zeros((N_PAD, 3), dtype=np.float32)
        buf[:N_CORE] = shards[i]
        in_maps.append({"vectors": buf})
    return in_maps


def kernel(vectors: np.ndarray) -> np.ndarray:
    res = run_spmd(make_in_maps(vectors))
    out = np.empty((N_TOTAL, K), dtype=np.float32)
    for i in range(N_CORES):
        out[i * N_CORE : (i + 1) * N_CORE] = res.results[i]["out"][:N_CORE]
    return out


# revision 8
# speedup vs baseline: 1.0481x; 1.0322x over previous
"""Trainium2 Bass kernel: monomials x^a y^b z^c (a+b+c <= 3) for N=2M points.

Data-parallel across 8 NeuronCores; each core gets N/8 = 250k points padded
to 128*F*T. The trivial columns (1, x, y, z) are assembled host-side; the
device computes only the 16 degree>=2 monomials, minimizing HBM write
traffic (the binding roofline: ~358 GB/s per core).

Per tile of 128 x F points:
  in-tile  it [P, F, 3]  (point-major interleaved x,y,z; contiguous load)
  out-tile ot [P, F, 16] (point-major; contiguous store)
Device cols: 0:x2 1:xy 2:xz 3:y2 4:yz 5:z2
             6:x3 7:x2y 8:x2z 9:xy2 10:xyz 11:xz2 12:y3 13:y2z 14:yz2 15:z3
DVE (fused, step-0 broadcast in0): deg2 = x*(x,y,z)->0:3, y*(y,z)->3:5,
  z*z->5; deg3 = x*cols0:6->6:12, y*cols3:6->12:15, z*col5->15.
ACT: issues out-DMAs. SP: in-DMAs, just-in-time (front-loading all inputs
delays the output stream: the input queue has strict priority on the SDMA
engines).

Raw bass (no Tile): this walrus rejects >1 sync-wait per instruction, so all
waits are standalone wait_ge ops. Every tile has its own input slot and
sem; output slots are BO-deep with per-slot sems (one DMA in flight per sem
keeps 16*n waits unambiguous).
"""

import sys
from contextlib import ExitStack

if "/opt/trn_rl_repo" not in sys.path:
    sys.path.insert(0, "/opt/trn_rl_repo")

import numpy as np
import concourse.bass as bass
import concourse.mybir as mybir
from concourse.bass_utils import run_bass_kernel_spmd

P = 128
K = 20
KD = 16  # device-computed columns (degree >= 2)
N_TOTAL = 2_000_000
N_CORES = 8
N_CORE = N_TOTAL // N_CORES  # 250_000
F = 245
T = 8
BO = 3
N_PAD = P * F * T  # 250_880

AF = mybir.ActivationFunctionType
F32 = mybir.dt.float32


def build(nc: bass.Bass, n_pts: int, f: int, bo: int = BO) -> bass.Bass:
    t_total = n_pts // (P * f)
    assert t_total * P * f == n_pts

    v = nc.declare_dram_parameter("vectors", [n_pts, 3], F32, isOutput=False)
    o = nc.declare_dram_parameter("out", [n_pts, KD], F32, isOutput=True)
    vr = v.rearrange("(t p f) c -> t p (f c)", p=P, f=f)
    orr = o.rearrange("(t p f) k -> t p (f k)", p=P, f=f)

    with ExitStack() as ctx:
        itb = ctx.enter_context(nc.sbuf_tensor("itb", [P, t_total * f * 3], F32))
        otb = ctx.enter_context(nc.sbuf_tensor("otb", [P, bo * f * KD], F32))
        s_in = [ctx.enter_context(nc.semaphore(f"s_in{i}")) for i in range(t_total)]
        s_out = [ctx.enter_context(nc.semaphore(f"s_out{i}")) for i in range(bo)]
        s_v = ctx.enter_context(nc.semaphore("s_v"))
        s_d = ctx.enter_context(nc.semaphore("s_d"))
        block = ctx.enter_context(nc.Block(no_gpsimd_drain=True))

        def it_view(t):
            return itb.ap()[:, t * f * 3 : (t + 1) * f * 3].rearrange(
                "p (f c) -> p f c", c=3
            )

        def ot_flat(s):
            return otb.ap()[:, s * f * KD : (s + 1) * f * KD]

        def ot_view(s):
            return ot_flat(s).rearrange("p (f k) -> p f k", k=KD)

        @block.sync
        def _(sync):
            for t in range(t_total):
                if t >= 3:
                    # JIT: issue once DVE has finished tile t-2, so the
                    # load lands while DVE chews tile t-1.
                    sync.wait_ge(s_v, t - 2)
                sync.dma_start(
                    out=itb.ap()[:, t * f * 3 : (t + 1) * f * 3], in_=vr[t]
                ).then_inc(s_in[t], 16)

        @block.vector
        def _(vector):
            for t in range(t_total):
                s = t % bo
                n_use = t // bo  # completed uses of this out slot
                itv = it_view(t)
                otv = ot_view(s)
                x = itv[:, :, 0:1]
                y = itv[:, :, 1:2]
                z = itv[:, :, 2:3]
                vector.wait_ge(s_in[t], 16)
                if t >= bo:
                    # WAR: out-DMA of the tile previously in this slot done
                    vector.wait_ge(s_out[s], 16 * n_use)
                nc.vector.tensor_mul(
                    otv[:, :, 0:3], x.broadcast_to([P, f, 3]), itv[:, :, 0:3]
                )
                nc.vector.tensor_mul(
                    otv[:, :, 3:5], y.broadcast_to([P, f, 2]), itv[:, :, 1:3]
                )
                nc.vector.tensor_mul(otv[:, :, 5:6], z, z).then_inc(s_d, 1)
                # Same-engine RAW: deg3 reads deg2 through the deep DVE
                # pipeline; wait for the deg2 writes to land.
                vector.wait_ge(s_d, t + 1)
                nc.vector.tensor_mul(
                    otv[:, :, 6:12], x.broadcast_to([P, f, 6]), otv[:, :, 0:6]
                )
                nc.vector.tensor_mul(
                    otv[:, :, 12:15], y.broadcast_to([P, f, 3]), otv[:, :, 3:6]
                )
                nc.vector.tensor_mul(otv[:, :, 15:16], z, otv[:, :, 5:6]).then_inc(
                    s_v, 1
                )

        @block.scalar
        def _(scalar):
            for t in range(t_total):
                s = t % bo
                scalar.wait_ge(s_v, t + 1)
                scalar.dma_start(out=orr[t], in_=ot_flat(s)).then_inc(s_out[s], 16)
            for s in range(bo):
                uses = len([t for t in range(t_total) if t % bo == s])
                if uses:
                    scalar.wait_ge(s_out[s], 16 * uses)

    return nc


_CACHE: dict[str, object] = {}


def _get_nc() -> bass.Bass:
    if "nc" not in _CACHE:
        nc = bass.Bass()
        build(nc, N_PAD, F, BO)
        _CACHE["nc"] = nc
    return _CACHE["nc"]  # type: ignore[return-value]


def run_spmd(in_maps, trace=False, **kw):
    return run_bass_kernel_spmd(
        _get_nc(), in_maps, core_ids=list(range(N_CORES)), trace=trace, **kw
    )


def make_in_maps(vectors: np.ndarray):
    vectors = np.ascontiguousarray(np.asarray(vectors, dtype=np.float32))
    assert vectors.shape == (N_TOTAL, 3)
    shards = vectors.reshape(N_CORES, N_CORE, 3)
    in_maps = []
    for i in range(N_CORES):
        buf = np.zeros((N_PAD, 3), dtype=np.float32)
        buf[:N_CORE] = shards[i]
        in_maps.append({"vectors": buf})
    return in_maps


def kernel(vectors: np.ndarray) -> np.ndarray:
    vec32 = np.ascontiguousarray(np.asarray(vectors, dtype=np.float32))
    res = run_spmd(make_in_maps(vec32))
    out = np.empty((N_TOTAL, K), dtype=np.float32)
    out[:, 0] = 1.0
    out[:, 1:4] = vec32  # degree-1 monomials are the input, exactly
    for i in range(N_CORES):
        out[i * N_CORE : (i + 1) * N_CORE, 4:] = res.results[i]["out"][:N_CORE]
    return out


# revision 9
# speedup vs baseline: 1.1498x; 1.0971x over previous
"""Trainium2 Bass kernel: monomials x^a y^b z^c (a+b+c <= 3) for N=2M points.

Data-parallel across 8 NeuronCores; each core gets N/8 = 250k points padded
to 128*F*T. The trivial columns (1, x, y, z) are assembled host-side; the
device computes only the 16 degree>=2 monomials, minimizing HBM write
traffic (the binding roofline: ~358 GB/s per core).

Per tile of 128 x F points:
  in-tile  it [P, F, 3]  (point-major interleaved x,y,z; contiguous load)
  out-tile ot [P, F, 16] (point-major; contiguous store)
Device cols: 0:x2 1:xy 2:xz 3:y2 4:yz 5:z2
             6:x3 7:x2y 8:x2z 9:xy2 10:xyz 11:xz2 12:y3 13:y2z 14:yz2 15:z3
DVE (fused, step-0 broadcast in0): deg2 = x*(x,y,z)->0:3, y*(y,z)->3:5,
  z*z->5; deg3 = x*cols0:6->6:12, y*cols3:6->12:15, z*col5->15.
ACT: issues out-DMAs. SP: in-DMAs, just-in-time (front-loading all inputs
delays the output stream: the input queue has strict priority on the SDMA
engines).

Raw bass (no Tile): this walrus rejects >1 sync-wait per instruction, so all
waits are standalone wait_ge ops. Every tile has its own input slot and
sem; output slots are BO-deep with per-slot sems (one DMA in flight per sem
keeps 16*n waits unambiguous).
"""

import sys
from contextlib import ExitStack

if "/opt/trn_rl_repo" not in sys.path:
    sys.path.insert(0, "/opt/trn_rl_repo")

import numpy as np
import concourse.bass as bass
import concourse.mybir as mybir
from concourse.bass_utils import run_bass_kernel_spmd

P = 128
K = 20
KD = 16  # device-computed columns (degree >= 2)
N_TOTAL = 2_000_000
N_CORES = 8
N_CORE = N_TOTAL // N_CORES  # 250_000
F = 245
T = 8
BO = 3
N_PAD = P * F * T  # 250_880

AF = mybir.ActivationFunctionType
F32 = mybir.dt.float32


def build(nc: bass.Bass, n_pts: int, f: int, bo: int = BO) -> bass.Bass:
    t_total = n_pts // (P * f)
    assert t_total * P * f == n_pts

    v = nc.declare_dram_parameter("vectors", [n_pts, 3], F32, isOutput=False)
    o = nc.declare_dram_parameter("out", [n_pts, KD], F32, isOutput=True)
    vr = v.rearrange("(t p f) c -> t p (f c)", p=P, f=f)
    orr = o.rearrange("(t p f) k -> t p (f k)", p=P, f=f)

    with ExitStack() as ctx:
        itb = ctx.enter_context(nc.sbuf_tensor("itb", [P, t_total * f * 3], F32))
        otb = ctx.enter_context(nc.sbuf_tensor("otb", [P, bo * f * KD], F32))
        s_in = [ctx.enter_context(nc.semaphore(f"s_in{i}")) for i in range(t_total)]
        s_out = [ctx.enter_context(nc.semaphore(f"s_out{i}")) for i in range(bo)]
        s_v = ctx.enter_context(nc.semaphore("s_v"))
        s_d = ctx.enter_context(nc.semaphore("s_d"))
        block = ctx.enter_context(nc.Block(no_gpsimd_drain=True))

        def it_view(t):
            return itb.ap()[:, t * f * 3 : (t + 1) * f * 3].rearrange(
                "p (f c) -> p f c", c=3
            )

        def ot_flat(s):
            return otb.ap()[:, s * f * KD : (s + 1) * f * KD]

        def ot_view(s):
            return ot_flat(s).rearrange("p (f k) -> p f k", k=KD)

        @block.sync
        def _(sync):
            # Front-load all input DMAs: the input queue has strict priority
            # over the output queue on the SDMA engines, so interleaving
            # punches holes in the output stream. Serialized streams both
            # run at the HBM ceiling; ins finish before the first out needs
            # the engines.
            for t in range(t_total):
                sync.dma_start(
                    out=itb.ap()[:, t * f * 3 : (t + 1) * f * 3], in_=vr[t]
                ).then_inc(s_in[t], 16)

        @block.vector
        def _(vector):
            for t in range(t_total):
                s = t % bo
                n_use = t // bo  # completed uses of this out slot
                itv = it_view(t)
                otv = ot_view(s)
                x = itv[:, :, 0:1]
                y = itv[:, :, 1:2]
                z = itv[:, :, 2:3]
                vector.wait_ge(s_in[t], 16)
                if t >= bo:
                    # WAR: out-DMA of the tile previously in this slot done
                    vector.wait_ge(s_out[s], 16 * n_use)
                nc.vector.tensor_mul(
                    otv[:, :, 0:3], x.broadcast_to([P, f, 3]), itv[:, :, 0:3]
                )
                nc.vector.tensor_mul(
                    otv[:, :, 3:5], y.broadcast_to([P, f, 2]), itv[:, :, 1:3]
                )
                nc.vector.tensor_mul(otv[:, :, 5:6], z, z).then_inc(s_d, 1)
                # Same-engine RAW: deg3 reads deg2 through the deep DVE
                # pipeline; wait for the deg2 writes to land.
                vector.wait_ge(s_d, t + 1)
                nc.vector.tensor_mul(
                    otv[:, :, 6:12], x.broadcast_to([P, f, 6]), otv[:, :, 0:6]
                )
                nc.vector.tensor_mul(
                    otv[:, :, 12:15], y.broadcast_to([P, f, 3]), otv[:, :, 3:6]
                )
                nc.vector.tensor_mul(otv[:, :, 15:16], z, otv[:, :, 5:6]).then_inc(
                    s_v, 1
                )

        @block.scalar
        def _(scalar):
            for t in range(t_total):
                s = t % bo
                scalar.wait_ge(s_v, t + 1)
                scalar.dma_start(out=orr[t], in_=ot_flat(s)).then_inc(s_out[s], 16)
            for s in range(bo):
                uses = len([t for t in range(t_total) if t % bo == s])
                if uses:
                    scalar.wait_ge(s_out[s], 16 * uses)

    return nc


_CACHE: dict[str, object] = {}


def _get_nc() -> bass.Bass:
    if "nc" not in _CACHE:
        nc = bass.Bass()
        build(nc, N_PAD, F, BO)
        _CACHE["nc"] = nc
    return _CACHE["nc"]  # type: ignore[return-value]


def run_spmd(in_maps, trace=False, **kw):
    return run_bass_kernel_spmd(
        _get_nc(), in_maps, core_ids=list(range(N_CORES)), trace=trace, **kw
    )


def make_in_maps(vectors: np.ndarray):
    vectors = np.ascontiguousarray(np.asarray(vectors, dtype=np.float32))
    assert vectors.shape == (N_TOTAL, 3)
    shards = vectors.reshape(N_CORES, N_CORE, 3)
    in_maps = []
    for i in range(N_CORES):
        buf = np.zeros((N_PAD, 3), dtype=np.float32)
        buf[:N_CORE] = shards[i]
        in_maps.append({"vectors": buf})
    return in_maps


def kernel(vectors: np.ndarray) -> np.ndarray:
    vec32 = np.ascontiguousarray(np.asarray(vectors, dtype=np.float32))
    res = run_spmd(make_in_maps(vec32))
    out = np.empty((N_TOTAL, K), dtype=np.float32)
    out[:, 0] = 1.0
    out[:, 1:4] = vec32  # degree-1 monomials are the input, exactly
    for i in range(N_CORES):
        out[i * N_CORE : (i + 1) * N_CORE, 4:] = res.results[i]["out"][:N_CORE]
    return out
